# revision 13
# baseline (speedup 1.0000x reference)
"""CBAM attention module (channel gate + spatial softmax attention) on 8 TRN2
NeuronCores, data-parallel over the batch dimension.

Reference computation (per sample b):
    m  = mean_n x[c, n];  mx = max_n x[c, n]
    gate = sigmoid(w2 @ (relu(w1 @ m) + relu(w1 @ mx)))          # (C,)
    x1 = gate[:, None] * x
    s  = sw0 * max_c x1 + sw1 * mean_c x1                        # (N,)
    s  = relu(A * s + Bconst)        # BatchNorm1d(1) eval, folded on host
    att = softmax_n(s)
    out = att[None, :] * x1

The kernel is HBM-bandwidth-bound, so the main trick is precision staging:
pass 1 streams the f32 input once (computing per-channel sum+max) and spills
an fp16 copy; passes 2 and 3 re-read the fp16 copy, and the output is written
fp16 (upcast to f32 on the host).  DRAM traffic per core drops from
3R+1W x f32 (512 MiB) to 1R f32 + 1W + 2R + 1W fp16 (384 MiB).  Tiles are
4096 wide so every DMA moves >= 8 KiB per partition line (the DGE is
descriptor-rate-bound below ~6 KiB).

Engine split per core (2 samples, DVE is the scarce engine at a flat
~112 G elem/s):
    pass 1: ScalarE Copy->fp16 with accum_out gives the fp16 spill AND the
            per-channel sum in one pass; GpSimd does the per-channel max
            (keeps DVE free).  Tiny MLP on TensorE -> gate.
    pass 2: TensorE matvec (gate-pair stationary, fp16) accumulates the
            gate-weighted c-sum in one PSUM bank (pieces at different
            partition offsets); one fused DVE scalar_tensor_tensor per chunk
            does tmax = max(gate*x16, tmax); TensorE transposes tmax blocks
            so DVE can finish the c-max.  Softmax over n in f32.
    pass 3: TensorE broadcasts att across partitions (ones stationary); one
            fused DVE scalar_tensor_tensor computes (x16*gate)*att -> out.
"""

import os
import numpy as np

B, C, N, RATIO = 16, 1024, 16384, 8
H = C // RATIO  # 128
BN_EPS = 1e-5
N_CORES = 8
BC = B // N_CORES  # samples per core

_cached = {}


def _build_nc(NT=4096, BC=BC, C=C, N=N, H=H):
    import concourse.bass as bass
    import concourse.bacc as bacc
    import concourse.mybir as mybir
    import concourse.tile as tile
    from concourse import masks
    from contextlib import ExitStack

    f32 = mybir.dt.float32
    f16 = mybir.dt.float16
    AF = mybir.ActivationFunctionType
    ALU = mybir.AluOpType
    X = mybir.AxisListType.X

    K = C // 128          # c-chunks (8)
    NJ = N // NT          # n-tiles per sample (passes 1/3)
    NT2 = min(4096, NT)   # pass-2 tile
    NJ2 = N // NT2
    BPT2 = NT2 // 128     # 128-blocks per pass-2 tile
    MVW = 512             # matvec piece width (one PSUM bank of f32)
    MV2 = NT2 // MVW      # matvec row-pieces per pass-2 tile
    NB = N // 128         # 128-blocks per sample (transpose-layout columns)
    TB = 8                # tmax blocks transposed per PSUM tile (1 fp16 bank)
    assert NB <= 128 and BPT2 % TB == 0 and 2 * MV2 <= 128

    nc = bacc.Bacc("TRN2", target_bir_lowering=False, debug=False,
                   num_devices=N_CORES)

    x = nc.dram_tensor("x", (BC, C, N), f32, kind="ExternalInput").ap()
    w1t = nc.dram_tensor("w1t", (C, H), f32, kind="ExternalInput").ap()
    w2t = nc.dram_tensor("w2t", (H, C), f32, kind="ExternalInput").ap()
    # params = [sw0, sw1/C, A, Bconst]
    params = nc.dram_tensor("params", (1, 4), f32, kind="ExternalInput").ap()
    x16 = nc.dram_tensor("x16", (BC, C, N), f16, kind="Internal").ap()
    out16 = nc.dram_tensor("out16", (BC, C, N), f16, kind="ExternalOutput").ap()
    att_dram = nc.dram_tensor("att_scratch", (BC, N), f16, kind="Internal").ap()
    cm_dram = nc.dram_tensor("cm_scratch", (BC, N), f32, kind="Internal").ap()

    with tile.TileContext(nc) as tc, ExitStack() as ctx:
        consts = ctx.enter_context(tc.tile_pool(name="consts", bufs=1))
        p1 = ctx.enter_context(tc.tile_pool(name="p1", bufs=2))
        p2 = ctx.enter_context(tc.tile_pool(name="p2", bufs=2))
        p3 = ctx.enter_context(tc.tile_pool(name="p3", bufs=2))
        small = ctx.enter_context(tc.tile_pool(name="small", bufs=2))
        psum = ctx.enter_context(tc.tile_pool(name="psum", bufs=2, space="PSUM"))

        # ---- constants ----
        identity = consts.tile([128, 128], f32)
        masks.make_identity(nc, identity)
        identity16 = consts.tile([128, 128], f16)
        masks.make_identity(nc, identity16)
        ones_row = consts.tile([1, 128], f32)
        nc.vector.memset(ones_row, 1.0)
        ones16 = consts.tile([1, 128], f16)
        nc.vector.memset(ones16, 1.0)
        params_sb = consts.tile([128, 4], f32)
        nc.sync.dma_start(out=params_sb, in_=params.to_broadcast((128, 4)))
        w1t_sb = consts.tile([128, K, H], f32)
        nc.sync.dma_start(out=w1t_sb, in_=w1t.rearrange("(k p) h -> p k h", p=128))
        w2t_sb = consts.tile([H, C], f32)
        nc.sync.dma_start(out=w2t_sb, in_=w2t)

        # ---- persistent stats ----
        mx_cols = consts.tile([128, BC, K, NJ], f32)
        sum_cols = consts.tile([128, BC, K, NJ], f32)
        stats = consts.tile([128, K, BC, 2], f32)   # per (k, b): [sum, max]
        # gate pair: PE stationary wants free >= 2.  DVE scalar operands
        # must be f32, so the gate is kept in both precisions.
        gate16 = consts.tile([128, K, BC, 2], f16)
        gate_sb = consts.tile([128, K, BC], f32)
        cx_t = consts.tile([128, BC, NB], f32)
        cmrows = consts.tile([NB, BC, 128], f32)

        xrs = [x[b].rearrange("(k p) n -> p k n", p=128) for b in range(BC)]
        x16rs = [x16[b].rearrange("(k p) n -> p k n", p=128) for b in range(BC)]
        outrs = [out16[b].rearrange("(k p) n -> p k n", p=128) for b in range(BC)]

        # ------- pass 1: per-channel sum & max over n; spill x16 -----------
        def p1_iter(b, j):
            for k in range(K):
                xin = p1.tile([128, NT], f32, tag="xin", bufs=3, name="xin")
                nc.sync.dma_start(out=xin, in_=xrs[b][:, k, j * NT:(j + 1) * NT])
                xc = p1.tile([128, NT], f16, tag="xc", bufs=2, name="xc")
                nc.scalar.activation(
                    out=xc, in_=xin, func=AF.Copy,
                    accum_out=sum_cols[:, b, k, j:j + 1])
                nc.sync.dma_start(out=x16rs[b][:, k, j * NT:(j + 1) * NT],
                                  in_=xc)
                nc.vector.reduce_max(out=mx_cols[:, b, k, j:j + 1],
                                     in_=xin, axis=X)

        # ------- MLP -> gate (per sample) ----------------------------------
        def mlp(b):
            nc.vector.reduce_sum(out=stats[:, :, b, 0:1],
                                 in_=sum_cols[:, b, :, :], axis=X)
            nc.vector.reduce_max(out=stats[:, :, b, 1:2],
                                 in_=mx_cols[:, b, :, :], axis=X)
            h_psum = psum.tile([H, 2], f32, tag="tp", name="h_psum")
            for k in range(K):
                nc.tensor.matmul(h_psum, lhsT=w1t_sb[:, k, :],
                                 rhs=stats[:, k, b, :],
                                 start=(k == 0), stop=(k == K - 1))
            hr = small.tile([H, 2], f32, tag="hr")
            nc.scalar.activation(out=hr[:, 0:1], in_=h_psum[:, 0:1],
                                 func=AF.Relu, scale=1.0 / N)
            nc.scalar.activation(out=hr[:, 1:2], in_=h_psum[:, 1:2],
                                 func=AF.Relu, scale=1.0)
            hsum = small.tile([H, 1], f32, tag="hsum")
            nc.vector.tensor_add(out=hsum, in0=hr[:, 0:1], in1=hr[:, 1:2])
            for k in range(K):
                g_psum = psum.tile([128, 1], f32, tag="tp", name="g_psum")
                nc.tensor.matmul(g_psum, lhsT=w2t_sb[:, k * 128:(k + 1) * 128],
                                 rhs=hsum, start=True, stop=True)
                nc.scalar.activation(out=gate_sb[:, k, b:b + 1],
                                     in_=g_psum, func=AF.Sigmoid)
                for i2 in range(2):
                    nc.scalar.activation(
                        out=gate16[:, k, b, i2:i2 + 1],
                        in_=g_psum, func=AF.Sigmoid)

        # ------- pass 2: x1 stats over c -----------------------------------
        def p2_iter(b, j):
            tmax = p2.tile([128, NT2], f16, tag="tmax", bufs=2, name="tmax")
            # gate-weighted c-sum: matmul PSUM outputs may only start at
            # partition 0/32/64, so each bank holds 3 row-pieces.
            nbank = (MV2 + 2) // 3
            cmb = [psum.tile([66, MVW], f32, tag=f"cmb{i}", bufs=1,
                             name=f"cmb{i}")
                   for i in range(nbank)]
            for k in range(K):
                xin = p2.tile([128, NT2], f16, tag="x2in", bufs=3, name="x2in")
                nc.sync.dma_start(out=xin,
                                  in_=x16rs[b][:, k, j * NT2:(j + 1) * NT2])
                for p8 in range(MV2):
                    base = 32 * (p8 % 3)
                    nc.tensor.matmul(
                        cmb[p8 // 3][base:base + 2, :],
                        lhsT=gate16[:, k, b, :],
                        rhs=xin[:, p8 * MVW:(p8 + 1) * MVW],
                        start=(k == 0), stop=(k == K - 1))
                if k == 0:
                    nc.vector.tensor_scalar(
                        out=tmax, in0=xin, scalar1=gate_sb[:, k, b:b + 1],
                        scalar2=None, op0=ALU.mult)
                else:
                    nc.vector.scalar_tensor_tensor(
                        out=tmax, in0=xin, scalar=gate_sb[:, k, b:b + 1],
                        in1=tmax, op0=ALU.mult, op1=ALU.max)
            # piece rows of cmb -> [1, MVW] stages -> DRAM (activation outputs
            # must be quadrant-aligned, so each piece stages separately)
            for p8 in range(MV2):
                base = 32 * (p8 % 3)
                cm_stage = small.tile([1, MVW], f32, tag="cmstage",
                                      name="cm_stage")
                nc.scalar.copy(out=cm_stage,
                               in_=cmb[p8 // 3][base:base + 1, :])
                n0 = j * NT2 + p8 * MVW
                nc.sync.dma_start(out=cm_dram[b:b + 1, n0:n0 + MVW],
                                  in_=cm_stage)
            # max over c: transpose 128x128 blocks, reduce over free dim
            for tb in range(BPT2 // TB):
                tp = psum.tile([128, TB, 128], f16, tag="tp", name="tp_t")
                for blk in range(TB):
                    c0 = (tb * TB + blk) * 128
                    nc.tensor.transpose(tp[:, blk, :], tmax[:, c0:c0 + 128],
                                        identity16)
                col = j * BPT2 + tb * TB
                nc.vector.reduce_max(out=cx_t[:, b, col:col + TB], in_=tp,
                                     axis=X)

        # ------- softmax over n (transpose layout, f32) --------------------
        def softmax(b):
            nc.sync.dma_start(
                out=cmrows[:, b, :],
                in_=cm_dram[b].rearrange("(jj p) -> jj p", p=128))
            cmt_psum = psum.tile([128, NB], f32, tag="tp", name="cmt_psum")
            nc.tensor.transpose(cmt_psum, cmrows[:, b, :],
                                identity[0:NB, 0:NB])
            s_t = small.tile([128, NB], f32, tag="st")
            # s = sw0 * cx + (sw1/C) * cm_sum
            nc.vector.tensor_scalar(out=s_t, in0=cmt_psum,
                                    scalar1=params_sb[:, 1:2], scalar2=None,
                                    op0=ALU.mult)
            tmp_t = small.tile([128, NB], f32, tag="st2")
            nc.vector.tensor_scalar(out=tmp_t, in0=cx_t[:, b, :],
                                    scalar1=params_sb[:, 0:1], scalar2=None,
                                    op0=ALU.mult)
            nc.vector.tensor_add(out=s_t, in0=s_t, in1=tmp_t)
            # BN (affine, host-folded) + relu
            nc.scalar.activation(out=s_t, in_=s_t, func=AF.Relu,
                                 scale=params_sb[:, 2:3],
                                 bias=params_sb[:, 3:4])
            # global max/sum over all partitions via PE transpose + ones
            # broadcast (a gpsimd partition_all_reduce would contend with the
            # pass-1 reduce stream).
            def preduce(col, op, nm):
                row_ps = psum.tile([1, 128], f32, tag="tp", name=nm + "_r")
                nc.tensor.transpose(row_ps, col, identity)
                scl = small.tile([1, 1], f32, tag=nm + "s", name=nm + "_s")
                nc.vector.tensor_reduce(out=scl, in_=row_ps, axis=X, op=op)
                rep_ps = psum.tile([128, 1], f32, tag="tp", name=nm + "_b")
                nc.tensor.matmul(rep_ps, lhsT=ones_row, rhs=scl,
                                 start=True, stop=True)
                rep = small.tile([128, 1], f32, tag=nm, name=nm)
                nc.scalar.copy(out=rep, in_=rep_ps)
                return rep
            colmax = small.tile([128, 1], f32, tag="cmax")
            nc.vector.reduce_max(out=colmax, in_=s_t, axis=X)
            gmax = preduce(colmax, ALU.max, "gmax")
            ngmax = small.tile([128, 1], f32, tag="ngmax")
            nc.vector.tensor_scalar(out=ngmax, in0=gmax, scalar1=-1.0,
                                    scalar2=None, op0=ALU.mult)
            e_t = small.tile([128, NB], f32, tag="et")
            sume = small.tile([128, 1], f32, tag="sume")
            nc.scalar.activation(out=e_t, in_=s_t, func=AF.Exp, bias=ngmax,
                                 scale=1.0, accum_out=sume)
            gsum = preduce(sume, ALU.add, "gsum")
            rinv = small.tile([128, 1], f32, tag="rinv")
            nc.vector.reciprocal(out=rinv, in_=gsum)
            att_t = small.tile([128, NB], f32, tag="attt")
            nc.vector.tensor_scalar(out=att_t, in0=e_t, scalar1=rinv,
                                    scalar2=None, op0=ALU.mult)
            # transpose-layout -> row-major (jj on partitions), fp16, store
            attt_psum = psum.tile([NB, 128], f32, tag="tp", name="attt_psum")
            nc.tensor.transpose(attt_psum, att_t, identity)
            att_rows = small.tile([NB, 128], f16, tag="attrows")
            nc.scalar.copy(out=att_rows, in_=attt_psum)
            nc.sync.dma_start(
                out=att_dram[b].rearrange("(jj p) -> jj p", p=128),
                in_=att_rows)

        # ------- pass 3: out = (x16 * gate) * att --------------------------
        def p3_iter(b, j):
            attp = small.tile([1, NT], f16, tag="attp", bufs=2, name="attp")
            nc.sync.dma_start(out=attp,
                              in_=att_dram[b:b + 1, j * NT:(j + 1) * NT])
            attr = p3.tile([128, NT], f16, tag="attr", bufs=2, name="attr")
            for p8 in range(NT // 512):
                bc_ps = psum.tile([128, 512], f32, tag="bc", name="bc_ps")
                nc.tensor.matmul(bc_ps, lhsT=ones16,
                                 rhs=attp[:, p8 * 512:(p8 + 1) * 512],
                                 start=True, stop=True)
                nc.vector.tensor_copy(out=attr[:, p8 * 512:(p8 + 1) * 512],
                                      in_=bc_ps)
            for k in range(K):
                xin = p3.tile([128, NT], f16, tag="x3in", bufs=3, name="x3in")
                nc.sync.dma_start(out=xin,
                                  in_=x16rs[b][:, k, j * NT:(j + 1) * NT])
                y = p3.tile([128, NT], f16, tag="y", bufs=2, name="y")
                nc.vector.scalar_tensor_tensor(
                    out=y, in0=xin, scalar=gate_sb[:, k, b:b + 1], in1=attr,
                    op0=ALU.mult, op1=ALU.mult)
                nc.sync.dma_start(out=outrs[b][:, k, j * NT:(j + 1) * NT],
                                  in_=y)

        # ------- emission schedule (software pipeline over the 2 samples) --
        # P1(b1) overlaps compute-heavy P2(b0); P2(b1) overlaps DMA-heavy
        # P3(b0), so the DMA queues always have a bandwidth-bound pass in
        # flight while a compute-bound pass runs.
        if BC == 2:
            for j in range(NJ):
                p1_iter(0, j)
            mlp(0)
            for j in range(max(NJ, NJ2)):
                if j < NJ:
                    p1_iter(1, j)
                if j < NJ2:
                    p2_iter(0, j)
            mlp(1)
            softmax(0)
            for j in range(max(NJ, NJ2)):
                if j < NJ2:
                    p2_iter(1, j)
                if j < NJ:
                    p3_iter(0, j)
            softmax(1)
            for j in range(NJ):
                p3_iter(1, j)
        else:
            for b in range(BC):
                for j in range(NJ):
                    p1_iter(b, j)
            for b in range(BC):
                mlp(b)
            for b in range(BC):
                for j in range(NJ2):
                    p2_iter(b, j)
                softmax(b)
            for b in range(BC):
                for j in range(NJ):
                    p3_iter(b, j)

    nc.compile()
    return nc


def _get_nc(NT=4096):
    key = ("nc", NT)
    if key not in _cached:
        _cached[key] = _build_nc(NT)
    return _cached[key]


def _host_params(sw, gamma, beta, running_mean, running_var):
    A = float(gamma[0]) / np.sqrt(float(running_var[0]) + BN_EPS)
    Bconst = float(beta[0]) - float(running_mean[0]) * A
    return np.array([[float(sw[0]), float(sw[1]) / C, A, Bconst]],
                    dtype=np.float32)


def _make_in_maps(x, w1, w2, sw, gamma, beta, running_mean, running_var):
    x = np.ascontiguousarray(np.asarray(x, dtype=np.float32))
    w1t = np.ascontiguousarray(np.asarray(w1, dtype=np.float32).T)
    w2t = np.ascontiguousarray(np.asarray(w2, dtype=np.float32).T)
    params = _host_params(np.asarray(sw), np.asarray(gamma), np.asarray(beta),
                          np.asarray(running_mean), np.asarray(running_var))
    in_maps = []
    for core in range(N_CORES):
        xs = np.ascontiguousarray(x[core * BC:(core + 1) * BC])
        in_maps.append({"x": xs, "w1t": w1t, "w2t": w2t, "params": params})
    return in_maps


def run_sharded(inputs, trace=False, NT=4096):
    """Run on all 8 cores; returns (out_full, BassKernelResults)."""
    from concourse.bass_utils import run_bass_kernel_spmd

    nc = _get_nc(NT)
    in_maps = _make_in_maps(**inputs)
    res = run_bass_kernel_spmd(nc, in_maps, core_ids=list(range(N_CORES)),
                               trace=trace)
    out = np.concatenate([r["out16"] for r in res.results], axis=0)
    return out.astype(np.float32), res


def kernel(**inputs) -> np.ndarray:
    out, _ = run_sharded(inputs, trace=False)
    return out
